# revision 33
# baseline (speedup 1.0000x reference)
"""NeuralCDE RK4 solver as a Bass/Tile kernel on 8 Trainium2 cores.

Data-parallel over batch: B=1024 -> 128 rows per core (one partition tile).
Wall time = 508 serial RK4 stages x per-stage chain latency, so everything
here is about shortening that chain (~3.7us/stage):
    kh    (DVE) : alpha*k^T   fp16 PSUM -> SBUF fp16
    mm1acc (PE) : h_ps slot += W1zH.T @ kh
    relu  (DVE) : hS = max(h_ps, 0)  (bias pre-added into PSUM via a rank-1
                  fp32 matmul off the critical path; bias1 = t*W1[0], needs
                  fp32 - fp16/f32r rounding of t*w accumulates a random-walk
                  trajectory error that breaks tolerance)
    mm2   (PE)  : f_ps[128b,512] = hS.T @ W2
    tanh  (ACT) : fS = tanh(f_psum)
    scan  (DVE) : custom fused op ANT_MULSCAN_K: prefix-sum of fS*g along
                  (h,c), fp32 out (fp16 prefixes would cancel catastrophically)
    diff  (DVE) : kn[128b,64h] = prefix[8h+7]-prefix[8h-1] (strided, via a
                  zero-padded leading group) - replaces the 1x tensor_reduce
    T     (PE)  : kn^T -> ksP (fp16 non-accumulating 1-pass transpose)
Big mm1 (W1z.T @ zT, f32r) per stage is pre-issued off-chain and emitted
AFTER mm2 so it never blocks the chain in the PE queue. RK4 sum: kn adds on
DVE off-chain; at s=3 the partial sum transposes through a (1/6)-scaled
identity (regular matmul - the transpose datapath ignores rhs values), and
one scalar_tensor_tensor yields delta-z^T feeding both the f32r state update
and next step's h correction. PE runs at the cold 1.2 GHz HAM rate: filler
matmuls do not lift the clock gate in this environment (verified), and fp8
DoubleRow mm2 breaks tolerance (6e-2 numpy-verified). State z^T lives in one
SBUF buffer [64, 128*128] f32r; slots stream out to DRAM as they finish.
Measured: 2195us (baseline) -> 1915us, rel err 1.2e-3 (tolerance 2e-2).
"""

import numpy as np
import ml_dtypes

import concourse.bacc as bacc
import concourse.bass as bass
import concourse.mybir as mybir
from concourse.tile import TileContext
from concourse.bass_utils import run_bass_kernel_spmd

F32 = mybir.dt.float32
F32R = mybir.dt.float32r
BF16 = mybir.dt.bfloat16
FP16 = mybir.dt.float16
B = 1024
L = 128
C_IN = 8
HID = 64
MLP_H = 128
INIT_H = 20
NSTEP = L - 1  # 127
NCORES = 8
BL = B // NCORES  # 128 batch rows per core
NF = HID * C_IN  # 512
NH = NF // 2  # 256 (half of the f block, h-split)

_CACHE: dict = {}


def _flags():
    import os
    return (
        int(os.environ.get("F16_BIG", "0")),   # big mm1 in fp16 instead of f32r
        int(os.environ.get("SPLIT", "0")),     # 0 none, 1 sym halves, 2 asym 128/384
        int(os.environ.get("BIAS_MM", "1")),   # fold bias1 into h PSUM via rank-1 matmul
        int(os.environ.get("FUSE", "1")),      # fused mul+prefix-scan custom DVE op
        int(os.environ.get("FSPLIT", "0")),    # half-split under the fused op
        int(os.environ.get("GEXP", "0")),      # host-expanded flat g, streamed
        int(os.environ.get("SEG", "0")),       # segmented scan: per-group reset
    )


_MULSCAN = None
_MULSCAN_SEG = None


def _get_mulscan():
    """Register (once) a fused custom DVE op: out = prefix-sum of in0*in1.

    Registered via the documented extension point (dve_ops.OPS append +
    sub-opcode map); the uops sha is computed from the lowered spec itself.
    """
    global _MULSCAN
    if _MULSCAN is not None:
        return _MULSCAN
    from concourse.dve_spec import Spec, Src0, Src1, scan, AluOp, lower
    from concourse.dve_spec import _has_src1 as has_src1
    from concourse import dve_ops as dops
    from concourse.dve_uop import DveOpSpec
    from concourse.dve_table_gen import dve_ver_for

    global _MULSCAN_SEG
    ver = dve_ver_for("TRN2")
    spec = Spec(body=scan(AluOp.ADD, Src0 * Src1))

    def reg(name, subdim):
        if name in dops._SUB_OPCODE_FOR_NAME:
            return next(o for o in dops.OPS if o.name == name)
        opcode = dops._CUSTOM_DVE_ROW_BASE + len(dops.OPS)
        tmp = DveOpSpec(
            name=name, opcode=opcode, uops=lower(spec, ver=ver),
            rd1_en=has_src1(spec),
        )
        op = dops.DveOp(name, spec, subdim=subdim, uops_sha={ver: tmp.sha(ver)})
        dops._SUB_OPCODE_FOR_NAME[name] = opcode
        dops.OPS.append(op)
        return op

    _MULSCAN = reg("ANT_MULSCAN_K", False)
    _MULSCAN_SEG = reg("ANT_MULSCAN_SEG", True)
    return _MULSCAN


def _build(nstep: int, with_b2: bool):
    import time as _time

    f16_big, split, bias_mm, fuse, fsplit, gexp, seg = _flags()
    if fuse:
        _get_mulscan()
    mulscan = (_MULSCAN_SEG if seg else _MULSCAN) if fuse else None
    BD = FP16 if f16_big else F32R
    t0 = _time.time()
    nc = bacc.Bacc()
    g_in = nc.dram_tensor("g", [BL, nstep * 3 * C_IN], FP16, kind="ExternalInput")
    gx_in = nc.dram_tensor(
        "gexp", [BL, nstep * 3 * NF if gexp else 1], FP16, kind="ExternalInput"
    )
    b1_in = nc.dram_tensor("bias1", [MLP_H, nstep * 3], F32, kind="ExternalInput")
    b1t_in = nc.dram_tensor("bias1t", [1, nstep * 3 * MLP_H], F32, kind="ExternalInput")
    ones32_in = nc.dram_tensor("ones32", [1, BL], F32, kind="ExternalInput")
    w1z_in = nc.dram_tensor("w1z", [HID, MLP_H], BD, kind="ExternalInput")
    w1zh_in = nc.dram_tensor("w1zh", [HID, MLP_H], FP16, kind="ExternalInput")
    w2_in = nc.dram_tensor("w2", [MLP_H, NF], FP16, kind="ExternalInput")
    b2_in = nc.dram_tensor("b2r", [1, NF], FP16, kind="ExternalInput")
    ones_in = nc.dram_tensor("onesr", [1, BL], FP16, kind="ExternalInput")
    id_in = nc.dram_tensor("ident", [BL, BL], F32 if seg else FP16,
                           kind="ExternalInput")
    id6_in = nc.dram_tensor("ident6", [BL, BL], F32 if seg else FP16,
                            kind="ExternalInput")
    z0t_in = nc.dram_tensor("z0t", [HID, BL], F32R, kind="ExternalInput")
    zs_out = nc.dram_tensor("zs", [HID, (nstep + 1) * BL], F32, kind="ExternalOutput")

    with TileContext(nc) as tc:
        with (
            tc.tile_pool(name="const", bufs=1) as cp,
            tc.tile_pool(name="zst", bufs=1) as zp,
            tc.tile_pool(name="hs", bufs=3) as hp,
            tc.tile_pool(name="fs", bufs=3) as fp,
            tc.tile_pool(name="us", bufs=3) as up,
            tc.tile_pool(name="ks", bufs=3) as kp,
            tc.tile_pool(name="an", bufs=2) as ap,
            tc.tile_pool(name="zs2", bufs=2) as zsp,
            tc.tile_pool(name="bt", bufs=2) as btp,
            tc.tile_pool(name="gx", bufs=2) as gxp,
            tc.tile_pool(name="kh", bufs=2) as khp,
            tc.tile_pool(name="ph", bufs=1, space="PSUM") as ph,
            tc.tile_pool(name="pf", bufs=2, space="PSUM") as pf,
            tc.tile_pool(name="pkt", bufs=3, space="PSUM") as pkt,
            tc.tile_pool(name="pa26", bufs=1, space="PSUM") as pa26,
        ):
            b1S = cp.tile([MLP_H, nstep * 3], F32)
            gS = cp.tile([BL, nstep * 3 * C_IN], FP16)
            w1zS = cp.tile([HID, MLP_H], BD)
            w1zH = cp.tile([HID, MLP_H], FP16)
            w2S = cp.tile([MLP_H, NF], FP16)
            b2S = cp.tile([1, NF], FP16)
            onesS = cp.tile([1, BL], FP16)
            idS = cp.tile([BL, BL], F32 if seg else FP16)
            ones32S = cp.tile([1, BL], F32)
            id6S = cp.tile([BL, BL], F32 if seg else FP16)
            KTD = F32 if seg else FP16
            zall = zp.tile([HID, (nstep + 1) * BL], F32R)
            _upad = 2 * C_IN if fsplit else C_IN
            uB = [cp.tile([BL, _upad + NF], F32, name=f"upre{i}")
                  for i in range(3)] if fuse else None
            if fuse:
                for t in uB:
                    nc.vector.memset(t[:, 0:C_IN], 0.0)
                    if fsplit:
                        nc.vector.memset(
                            t[:, C_IN + NH : 2 * C_IN + NH], 0.0
                        )

            def gx_tile(step):
                t = gxp.tile([BL, 3 * NF], FP16, tag="gx", name="gx")
                nc.sync.dma_start(
                    out=t[:],
                    in_=gx_in[:, step * 3 * NF : (step + 1) * 3 * NF],
                )
                return t

            gx_cur = gx_tile(0) if gexp else None
            gx_next = None

            nc.sync.dma_start(out=gS[:], in_=g_in[:])
            nc.sync.dma_start(out=b1S[:], in_=b1_in[:])
            nc.sync.dma_start(out=w1zS[:], in_=w1z_in[:])
            nc.sync.dma_start(out=w1zH[:], in_=w1zh_in[:])
            nc.sync.dma_start(out=w2S[:], in_=w2_in[:])
            nc.sync.dma_start(out=b2S[:], in_=b2_in[:])
            nc.sync.dma_start(out=onesS[:], in_=ones_in[:])
            nc.sync.dma_start(out=idS[:], in_=id_in[:])
            nc.sync.dma_start(out=ones32S[:], in_=ones32_in[:])
            nc.sync.dma_start(out=id6S[:], in_=id6_in[:])
            nc.sync.dma_start(out=zall[:, 0:BL], in_=z0t_in[:])
            nc.sync.dma_start(out=zs_out[:, 0:BL], in_=z0t_in[:].bitcast(F32))

            # h PSUM: one bank, 4 rotating [128,128] stage slots. Emission
            # order guarantees at most one open accumulation group at a time.
            hP = ph.tile([MLP_H, 4 * BL], F32, name="hP")

            def h_slot(step, s):
                i = (4 * step + s) % 4
                return hP[:, i * BL : (i + 1) * BL]

            CLS = (0, 1, 1, 2)
            KH_A = (1.0 / 6.0, 0.5, 0.25, 0.5)

            def zT_sl(step):
                return zall[:, step * BL : (step + 1) * BL]

            def bt_tile(step):
                t = btp.tile([1, 3 * MLP_H], F32, tag="bt", name="bt")
                nc.sync.dma_start(
                    out=t[:],
                    in_=b1t_in[:, step * 3 * MLP_H : (step + 1) * 3 * MLP_H],
                )
                return t

            bt_cur = bt_tile(0) if bias_mm else None
            bt_next = None

            # step 0 slice-0 big (no k correction at the very first stage)
            nc.tensor.matmul(
                h_slot(0, 0), lhsT=w1zS[:], rhs=zT_sl(0), start=True,
                stop=not bias_mm,
            )
            if bias_mm:
                nc.tensor.matmul(
                    h_slot(0, 0), lhsT=bt_cur[:, 0:MLP_H], rhs=ones32S[:],
                    start=False, stop=True, skip_group_check=True,
                )

            acc_nat = None   # kn1+kn2 (+kn3) natural-layout partial RK4 sum
            acc2T6 = None    # (acc_nat at s=2).T / 6 in PSUM
            zsum = None      # zT + acc2T6, f32r (state update staging)
            kt4P = None      # k4~.T PSUM
            ksP = None       # k~_s.T PSUM for next stage's kh
            kh0 = None       # delta-z^T fp16 (next step's h correction)

            for step in range(nstep):
                zT = zT_sl(step)
                if bias_mm and step + 1 < nstep:
                    bt_next = bt_tile(step + 1)
                if gexp and step + 1 < nstep:
                    gx_next = gx_tile(step + 1)
                for s in range(4):
                    col = step * 3 + CLS[s]
                    has_b = not (step == 0 and s == 0)
                    # ---- kh for this stage ----
                    if has_b:
                        kh = khp.tile([HID, BL], FP16, tag="kh", name="kh")
                        if s == 0:
                            # kh0 = (k4~.T)/6 + acc2T6 = delta-z^T
                            nc.vector.scalar_tensor_tensor(
                                out=kh[:],
                                in0=kt4P[:],
                                scalar=1.0 / 6.0,
                                in1=acc2T6[:],
                                op0=mybir.AluOpType.mult,
                                op1=mybir.AluOpType.add,
                            )
                            # state update z_step = zsum + (k4~.T)/6, f32r
                            nc.vector.scalar_tensor_tensor(
                                out=zT,
                                in0=kt4P[:],
                                scalar=1.0 / 6.0,
                                in1=zsum[:],
                                op0=mybir.AluOpType.mult,
                                op1=mybir.AluOpType.add,
                            )
                            nc.sync.dma_start(
                                out=zs_out[:, step * BL : (step + 1) * BL],
                                in_=zT.bitcast(F32),
                            )
                        else:
                            nc.vector.tensor_scalar_mul(kh[:], ksP[:], KH_A[s])
                        nc.tensor.matmul(
                            h_slot(step, s), lhsT=w1zH[:], rhs=kh[:],
                            start=False, stop=True,
                        )
                    # ---- relu (bias already in PSUM when bias_mm) ----
                    hS = hp.tile([MLP_H, BL], FP16, tag="hs")
                    if bias_mm:
                        nc.vector.tensor_scalar_max(hS[:], h_slot(step, s), 0.0)
                    else:
                        nc.vector.tensor_scalar(
                            hS[:], h_slot(step, s), b1S[:, col : col + 1], 0.0,
                            op0=mybir.AluOpType.add, op1=mybir.AluOpType.max,
                        )
                    # ---- mm2 (+ optional bias2), h-split halves ----
                    f_ps = pf.tile([BL, NF], F32, tag="fps")
                    if with_b2:
                        nc.tensor.matmul(
                            f_ps[:], lhsT=onesS[:], rhs=b2S[:],
                            start=True, stop=False,
                        )
                    if fuse and fsplit:
                        halves = ((0, NH), (NH, NF))
                    elif split == 2:
                        halves = ((0, 128), (128, NF))
                    elif split == 1:
                        halves = ((0, NH), (NH, NF))
                    else:
                        halves = ((0, NF),)
                    for (lo, hi) in halves:
                        nc.tensor.matmul(
                            f_ps[:, lo:hi], lhsT=hS[:], rhs=w2S[:, lo:hi],
                            start=not with_b2, stop=True,
                        )
                    # ---- big mm1 for the next stage slot (off chain) ----
                    if s < 3:
                        nc.tensor.matmul(
                            h_slot(step, s + 1), lhsT=w1zS[:], rhs=zT,
                            start=True, stop=False,
                        )
                        if bias_mm:
                            ncol = CLS[s + 1]
                            nc.tensor.matmul(
                                h_slot(step, s + 1),
                                lhsT=bt_cur[:, ncol * MLP_H : (ncol + 1) * MLP_H],
                                rhs=ones32S[:],
                                start=False, stop=False, skip_group_check=True,
                            )
                    elif step + 1 < nstep:
                        nc.tensor.matmul(
                            h_slot(step + 1, 0), lhsT=w1zS[:], rhs=zT,
                            start=True, stop=False,
                        )
                        if bias_mm:
                            nc.tensor.matmul(
                                h_slot(step + 1, 0),
                                lhsT=bt_next[:, 0:MLP_H],
                                rhs=ones32S[:],
                                start=False, stop=False, skip_group_check=True,
                            )
                    # ---- tanh / mul / reduce, pipelined across halves ----
                    fS = fp.tile([BL, NF], FP16, tag="fs")
                    u = None if fuse else up.tile([BL, NF], FP16, tag="u")
                    kn = kp.tile([BL, HID], FP16, tag="kn")
                    for (lo, hi) in halves:
                        nc.scalar.activation(
                            fS[:, lo:hi], f_ps[:, lo:hi],
                            mybir.ActivationFunctionType.Tanh,
                        )
                    if fuse and seg:
                        uP = uB[(4 * step + s) % 3]
                        f3 = fS[:].rearrange("p (h c) -> p h c", c=C_IN)
                        gv = (
                            gS[:, col * C_IN : (col + 1) * C_IN]
                            .unsqueeze(1)
                            .broadcast_to((BL, HID, C_IN))
                        )
                        scan_out = uP[:, 0:NF].rearrange(
                            "p (h c) -> p h c", c=C_IN
                        )
                        nc.vector._custom_dve(
                            mulscan, out=scan_out, in0=f3, in1=gv
                        )
                        # group-end prefix IS kn (accumulator reseeds per
                        # 8-elem page); strided fp32 view, no diff op
                        kn_ap = (
                            uP[:, 0:NF]
                            .rearrange("p (h c) -> p h c", c=C_IN)
                            [:, :, C_IN - 1 : C_IN]
                        )
                        kn2d = kn_ap.rearrange("p h one -> p (h one)")
                        kn = None
                    elif fuse:
                        uP = uB[(4 * step + s) % 3]
                        v = uP[:].rearrange("p (g c) -> p g c", c=C_IN)
                        cg = CLS[s]
                        for hi_, (lo, hi) in enumerate(halves):
                            nh = hi - lo
                            uoff = C_IN + lo + hi_ * (C_IN if fsplit else 0)
                            scan_out = uP[:, uoff : uoff + nh].rearrange(
                                "p (h c) -> p h c", c=C_IN
                            )
                            f3 = fS[:, lo:hi].rearrange(
                                "p (h c) -> p h c", c=C_IN
                            )
                            if gexp:
                                gv = gx_cur[
                                    :, cg * NF + lo : cg * NF + hi
                                ].rearrange("p (h c) -> p h c", c=C_IN)
                            else:
                                gv = (
                                    gS[:, col * C_IN : (col + 1) * C_IN]
                                    .unsqueeze(1)
                                    .broadcast_to((BL, nh // C_IN, C_IN))
                                )
                            nc.vector._custom_dve(
                                mulscan, out=scan_out, in0=f3, in1=gv
                            )
                            # kn[h] = prefix[8h+7]-prefix[8(h-1)+7] via the
                            # zero-padded leading group of each half
                            glo = uoff // C_IN - 1
                            hlo, hn = lo // C_IN, nh // C_IN
                            with nc.allow_low_precision("k diff"):
                                nc.vector.tensor_tensor(
                                    out=kn[:, hlo : hlo + hn].unsqueeze(2),
                                    in0=v[:, glo + 1 : glo + 1 + hn,
                                          C_IN - 1 : C_IN],
                                    in1=v[:, glo : glo + hn,
                                          C_IN - 1 : C_IN],
                                    op=mybir.AluOpType.subtract,
                                )
                    else:
                        for (lo, hi) in halves:
                            hlo, hhi = lo // C_IN, hi // C_IN
                            f3 = fS[:, lo:hi].rearrange("p (h c) -> p h c", c=C_IN)
                            u3 = u[:, lo:hi].rearrange("p (h c) -> p h c", c=C_IN)
                            gv = (
                                gS[:, col * C_IN : (col + 1) * C_IN]
                                .unsqueeze(1)
                                .broadcast_to((BL, hhi - hlo, C_IN))
                            )
                            nc.vector.tensor_tensor(
                                out=u3, in0=f3, in1=gv, op=mybir.AluOpType.mult
                            )
                            with nc.allow_low_precision("k reduce"):
                                nc.vector.tensor_reduce(
                                    kn[:, hlo:hhi], u3, axis=mybir.AxisListType.X,
                                    op=mybir.AluOpType.add,
                                )
                    # ---- transpose + RK4 bookkeeping ----
                    tr_lhs = kn2d if (fuse and seg) else kn[:]
                    if s < 3:
                        ksP = pkt.tile([HID, BL], KTD, tag="kt", name="kt")
                        nc.tensor.matmul(
                            ksP[:], lhsT=tr_lhs, rhs=idS[:], is_transpose=True,
                            start=True, stop=True,
                        )
                        if s == 0:
                            # acc3 holds the rank-3 AP form for seg mode
                            acc3 = kn_ap if (fuse and seg) else None
                            acc_nat = None if (fuse and seg) else kn
                        else:
                            acc_new = ap.tile([BL, HID], KTD, tag="an",
                                              name="an")
                            if fuse and seg:
                                nc.vector.tensor_tensor(
                                    out=acc_new[:].unsqueeze(2),
                                    in0=acc3,
                                    in1=kn_ap,
                                    op=mybir.AluOpType.add,
                                )
                                acc3 = acc_new[:].unsqueeze(2)
                            else:
                                nc.vector.tensor_tensor(
                                    out=acc_new[:], in0=acc_nat[:], in1=kn[:],
                                    op=mybir.AluOpType.add,
                                )
                            acc_nat = acc_new
                        if s == 2:
                            # off-chain: acc2T6 = acc_nat.T / 6 (via scaled
                            # identity), staged to SBUF so the s=0 STT has a
                            # single PSUM operand; zsum = zT + acc2T6
                            a26P = pa26.tile([HID, BL], F32, tag="a26",
                                             name="a26")
                            a26_lhs = acc_nat[:]
                            # regular matmul: transpose datapath would ignore
                            # the scaled identity's values
                            nc.tensor.matmul(
                                a26P[:], lhsT=a26_lhs, rhs=id6S[:],
                                start=True, stop=True,
                            )
                            acc2T6 = zsp.tile([HID, BL], FP16, tag="a26s",
                                              name="a26s")
                            nc.vector.tensor_copy(acc2T6[:], a26P[:])
                            zsum = zsp.tile([HID, BL], F32R, tag="zsum",
                                            name="zsum")
                            nc.vector.tensor_tensor(
                                out=zsum[:], in0=zT, in1=acc2T6[:],
                                op=mybir.AluOpType.add,
                            )
                    else:
                        kt4P = pkt.tile([HID, BL], KTD, tag="kt", name="kt4")
                        nc.tensor.matmul(
                            kt4P[:], lhsT=tr_lhs, rhs=idS[:],
                            is_transpose=True, start=True, stop=True,
                        )
                        bt_cur = bt_next
                        gx_cur = gx_next

            # final z update (last grid point)
            nc.vector.scalar_tensor_tensor(
                out=zT_sl(nstep),
                in0=kt4P[:],
                scalar=1.0 / 6.0,
                in1=zsum[:],
                op0=mybir.AluOpType.mult,
                op1=mybir.AluOpType.add,
            )
            nc.sync.dma_start(
                out=zs_out[:, nstep * BL : (nstep + 1) * BL],
                in_=zT_sl(nstep).bitcast(F32),
            )
    import sys

    print(f"[kernel] tile trace+schedule: {_time.time()-t0:.1f}s", file=sys.stderr)
    t1 = _time.time()
    nc.finalize()
    print(f"[kernel] finalize: {_time.time()-t1:.1f}s", file=sys.stderr)
    return nc


def _get_nc(nstep: int, with_b2: bool):
    key = (nstep, with_b2) + _flags()
    if key not in _CACHE:
        _CACHE[key] = _build(nstep, with_b2)
    return _CACHE[key]


def _host_prep(coeffs, Wi1, bi1, Wi2, bi2, W1, b1, W2, b2, nstep: int):
    coeffs = np.asarray(coeffs, dtype=np.float32)
    a = coeffs[:, :, 0:8]
    b = coeffs[:, :, 8:16]
    c = coeffs[:, :, 16:24]
    d = coeffs[:, :, 24:32]

    X0 = a[:, 0]
    z0 = np.tanh(
        np.maximum(X0 @ Wi1 + bi1, 0.0).astype(np.float32) @ Wi2 + bi2
    ).astype(np.float32)

    g = np.empty((B, nstep, 3, C_IN), dtype=np.float32)
    g[:, :, 0] = b[:, :nstep]
    g[:, :, 1] = 2.0 * b[:, :nstep] + 2.0 * c[:, :nstep] + 1.5 * d[:, :nstep]
    last = NSTEP - 1  # 126 in full problem
    for i in range(nstep):
        if i < last:
            g[:, i, 2] = b[:, i + 1]
        else:
            g[:, i, 2] = b[:, i] + 2.0 * c[:, i] + 3.0 * d[:, i]
    g16 = g.reshape(B, nstep * 3 * C_IN).astype(np.float16)
    gexp_on = _flags()[5]
    gx16 = (np.broadcast_to(
        g.astype(np.float16)[:, :, :, None, :], (B, nstep, 3, HID, C_IN)
    ).reshape(B, nstep * 3 * NF) if gexp_on else None)

    tcols = np.empty((nstep, 3), dtype=np.float32)
    tcols[:, 0] = np.arange(nstep, dtype=np.float32)
    tcols[:, 1] = tcols[:, 0] + 0.5
    tcols[:, 2] = tcols[:, 0] + 1.0
    bias1 = (
        b1[None, None, :] + tcols[:, :, None] * W1[0][None, None, :]
    ).astype(np.float32)
    bias1 = bias1.reshape(nstep * 3, MLP_H).T.copy()  # [128, nstep*3]

    f16_big = _flags()[0]
    bias1t = np.ascontiguousarray(bias1.T.reshape(1, -1))
    shared = {
        "bias1": bias1,
        "bias1t": bias1t,
        "ones32": np.ones((1, BL), dtype=np.float32),
        "w1z": np.ascontiguousarray(
            W1[1:], dtype=(np.float16 if f16_big else np.float32)
        ),
        "w1zh": np.ascontiguousarray(W1[1:], dtype=np.float16),
        "w2": np.ascontiguousarray(W2, dtype=np.float16),
        "b2r": np.ascontiguousarray(b2[None, :], dtype=np.float16),
        "onesr": np.ones((1, BL), dtype=np.float16),
        "ident": np.eye(
            BL, dtype=(np.float32 if _flags()[6] else np.float16)
        ),
        "ident6": (np.eye(BL) / 6.0).astype(
            np.float32 if _flags()[6] else np.float16
        ),
    }
    in_maps = []
    for core in range(NCORES):
        sl = slice(core * BL, (core + 1) * BL)
        m = dict(shared)
        m["g"] = np.ascontiguousarray(g16[sl])
        m["gexp"] = (np.ascontiguousarray(gx16[sl]) if gexp_on
                     else np.zeros((BL, 1), dtype=np.float16))
        m["z0t"] = np.ascontiguousarray(z0[sl].T)
        in_maps.append(m)
    return in_maps, z0


def kernel(coeffs, Wi1, bi1, Wi2, bi2, W1, b1, W2, b2, _nstep: int = NSTEP,
           _trace: bool = False):
    import time as _time
    import sys

    nstep = _nstep
    with_b2 = bool(np.any(np.asarray(b2)))
    nc = _get_nc(nstep, with_b2)
    in_maps, _ = _host_prep(
        coeffs, Wi1, bi1, Wi2, bi2, W1, b1, W2, b2, nstep
    )
    t0 = _time.time()
    res = run_bass_kernel_spmd(nc, in_maps, list(range(NCORES)), trace=_trace)
    print(f"[kernel] spmd run (compile+exec): {_time.time()-t0:.1f}s", file=sys.stderr)
    out = np.empty((B, nstep + 1, HID), dtype=np.float32)
    for core in range(NCORES):
        zs = res.results[core]["zs"].reshape(HID, nstep + 1, BL)
        out[core * BL : (core + 1) * BL] = zs.transpose(2, 1, 0)
    if _trace:
        kernel.last_results = res
    return out


# revision 34
# speedup vs baseline: 1.0141x; 1.0141x over previous
"""NeuralCDE RK4 solver as a Bass/Tile kernel on 8 Trainium2 cores.

Data-parallel over batch: B=1024 -> 128 rows per core (one partition tile).
Wall time = 508 serial RK4 stages x per-stage chain latency, so everything
here is about shortening that chain (~3.7us/stage):
    kh    (DVE) : alpha*k^T   fp16 PSUM -> SBUF fp16
    mm1acc (PE) : h_ps slot += W1zH.T @ kh
    relu  (DVE) : hS = max(h_ps, 0)  (bias pre-added into PSUM via a rank-1
                  fp32 matmul off the critical path; bias1 = t*W1[0], needs
                  fp32 - fp16/f32r rounding of t*w accumulates a random-walk
                  trajectory error that breaks tolerance)
    mm2   (PE)  : f_ps[128b,512] = hS.T @ W2
    tanh  (ACT) : fS = tanh(f_psum)
    scan  (DVE) : custom fused op ANT_MULSCAN_K: prefix-sum of fS*g along
                  (h,c), fp32 out (fp16 prefixes would cancel catastrophically)
    diff  (DVE) : kn[128b,64h] = prefix[8h+7]-prefix[8h-1] (strided, via a
                  zero-padded leading group) - replaces the 1x tensor_reduce
    T     (PE)  : kn^T -> ksP (fp16 non-accumulating 1-pass transpose)
Big mm1 (W1z.T @ zT, f32r) per stage is pre-issued off-chain and emitted
AFTER mm2 so it never blocks the chain in the PE queue. RK4 sum: kn adds on
DVE off-chain; at s=3 the partial sum transposes through a (1/6)-scaled
identity (regular matmul - the transpose datapath ignores rhs values), and
one scalar_tensor_tensor yields delta-z^T feeding both the f32r state update
and next step's h correction. PE runs at the cold 1.2 GHz HAM rate: filler
matmuls do not lift the clock gate in this environment (verified), and fp8
DoubleRow mm2 breaks tolerance (6e-2 numpy-verified). State z^T lives in one
SBUF buffer [64, 128*128] f32r; slots stream out to DRAM as they finish.
Measured: 2195us (baseline) -> 1915us, rel err 1.2e-3 (tolerance 2e-2).
"""

import numpy as np
import ml_dtypes

import concourse.bacc as bacc
import concourse.bass as bass
import concourse.mybir as mybir
from concourse.tile import TileContext
from concourse.bass_utils import run_bass_kernel_spmd

F32 = mybir.dt.float32
F32R = mybir.dt.float32r
BF16 = mybir.dt.bfloat16
FP16 = mybir.dt.float16
B = 1024
L = 128
C_IN = 8
HID = 64
MLP_H = 128
INIT_H = 20
NSTEP = L - 1  # 127
NCORES = 8
BL = B // NCORES  # 128 batch rows per core
NF = HID * C_IN  # 512
NH = NF // 2  # 256 (half of the f block, h-split)

_CACHE: dict = {}


def _flags():
    import os
    return (
        int(os.environ.get("F16_BIG", "0")),   # big mm1 in fp16 instead of f32r
        int(os.environ.get("SPLIT", "0")),     # 0 none, 1 sym halves, 2 asym 128/384
        int(os.environ.get("BIAS_MM", "1")),   # fold bias1 into h PSUM via rank-1 matmul
        int(os.environ.get("FUSE", "1")),      # fused mul+prefix-scan custom DVE op
        int(os.environ.get("FSPLIT", "0")),    # half-split under the fused op
        int(os.environ.get("GEXP", "0")),      # host-expanded flat g, streamed
        int(os.environ.get("SEG", "0")),       # segmented scan: per-group reset
        int(os.environ.get("SC0", "1")),       # stride-0 scan out (compact kn')
    )


_MULSCAN = None
_MULSCAN_SEG = None


def _get_mulscan():
    """Register (once) a fused custom DVE op: out = prefix-sum of in0*in1.

    Registered via the documented extension point (dve_ops.OPS append +
    sub-opcode map); the uops sha is computed from the lowered spec itself.
    """
    global _MULSCAN
    if _MULSCAN is not None:
        return _MULSCAN
    from concourse.dve_spec import Spec, Src0, Src1, scan, AluOp, lower
    from concourse.dve_spec import _has_src1 as has_src1
    from concourse import dve_ops as dops
    from concourse.dve_uop import DveOpSpec
    from concourse.dve_table_gen import dve_ver_for

    global _MULSCAN_SEG
    ver = dve_ver_for("TRN2")
    spec = Spec(body=scan(AluOp.ADD, Src0 * Src1))

    def reg(name, subdim):
        if name in dops._SUB_OPCODE_FOR_NAME:
            return next(o for o in dops.OPS if o.name == name)
        opcode = dops._CUSTOM_DVE_ROW_BASE + len(dops.OPS)
        tmp = DveOpSpec(
            name=name, opcode=opcode, uops=lower(spec, ver=ver),
            rd1_en=has_src1(spec),
        )
        op = dops.DveOp(name, spec, subdim=subdim, uops_sha={ver: tmp.sha(ver)})
        dops._SUB_OPCODE_FOR_NAME[name] = opcode
        dops.OPS.append(op)
        return op

    _MULSCAN = reg("ANT_MULSCAN_K", False)
    _MULSCAN_SEG = reg("ANT_MULSCAN_SEG", True)
    return _MULSCAN


def _build(nstep: int, with_b2: bool):
    import time as _time

    f16_big, split, bias_mm, fuse, fsplit, gexp, seg, sc0 = _flags()
    if fuse:
        _get_mulscan()
    mulscan = (_MULSCAN_SEG if seg else _MULSCAN) if fuse else None
    BD = FP16 if f16_big else F32R
    t0 = _time.time()
    nc = bacc.Bacc()
    g_in = nc.dram_tensor("g", [BL, nstep * 3 * C_IN], FP16, kind="ExternalInput")
    gx_in = nc.dram_tensor(
        "gexp", [BL, nstep * 3 * NF if gexp else 1], FP16, kind="ExternalInput"
    )
    b1_in = nc.dram_tensor("bias1", [MLP_H, nstep * 3], F32, kind="ExternalInput")
    b1t_in = nc.dram_tensor("bias1t", [1, nstep * 3 * MLP_H], F32, kind="ExternalInput")
    ones32_in = nc.dram_tensor("ones32", [1, BL], F32, kind="ExternalInput")
    w1z_in = nc.dram_tensor("w1z", [HID, MLP_H], BD, kind="ExternalInput")
    w1zh_in = nc.dram_tensor("w1zh", [HID, MLP_H], FP16, kind="ExternalInput")
    w2_in = nc.dram_tensor("w2", [MLP_H, NF], FP16, kind="ExternalInput")
    b2_in = nc.dram_tensor("b2r", [1, NF], FP16, kind="ExternalInput")
    ones_in = nc.dram_tensor("onesr", [1, BL], FP16, kind="ExternalInput")
    id_in = nc.dram_tensor("ident", [BL, BL], F32 if seg else FP16,
                           kind="ExternalInput")
    id6_in = nc.dram_tensor("ident6", [BL, BL], F32 if seg else FP16,
                            kind="ExternalInput")
    z0t_in = nc.dram_tensor("z0t", [HID, BL], F32R, kind="ExternalInput")
    zs_out = nc.dram_tensor("zs", [HID, (nstep + 1) * BL], F32, kind="ExternalOutput")

    with TileContext(nc) as tc:
        with (
            tc.tile_pool(name="const", bufs=1) as cp,
            tc.tile_pool(name="zst", bufs=1) as zp,
            tc.tile_pool(name="hs", bufs=3) as hp,
            tc.tile_pool(name="fs", bufs=3) as fp,
            tc.tile_pool(name="us", bufs=3) as up,
            tc.tile_pool(name="ks", bufs=3) as kp,
            tc.tile_pool(name="an", bufs=2) as ap,
            tc.tile_pool(name="zs2", bufs=2) as zsp,
            tc.tile_pool(name="bt", bufs=2) as btp,
            tc.tile_pool(name="gx", bufs=2) as gxp,
            tc.tile_pool(name="kh", bufs=2) as khp,
            tc.tile_pool(name="ph", bufs=1, space="PSUM") as ph,
            tc.tile_pool(name="pf", bufs=2, space="PSUM") as pf,
            tc.tile_pool(name="pkt", bufs=3, space="PSUM") as pkt,
            tc.tile_pool(name="pa26", bufs=1, space="PSUM") as pa26,
        ):
            b1S = cp.tile([MLP_H, nstep * 3], F32)
            gS = cp.tile([BL, nstep * 3 * C_IN], FP16)
            w1zS = cp.tile([HID, MLP_H], BD)
            w1zH = cp.tile([HID, MLP_H], FP16)
            w2S = cp.tile([MLP_H, NF], FP16)
            b2S = cp.tile([1, NF], FP16)
            onesS = cp.tile([1, BL], FP16)
            idS = cp.tile([BL, BL], F32 if seg else FP16)
            ones32S = cp.tile([1, BL], F32)
            id6S = cp.tile([BL, BL], F32 if seg else FP16)
            KTD = F32 if seg else FP16
            zall = zp.tile([HID, (nstep + 1) * BL], F32R)
            _upad = 2 * C_IN if fsplit else C_IN
            uB = [cp.tile([BL, _upad + NF], F32, name=f"upre{i}")
                  for i in range(3)] if (fuse and not sc0) else None
            knB = [cp.tile([BL, 1 + HID], F32, name=f"knp{i}")
                   for i in range(3)] if (fuse and sc0) else None
            if fuse and not sc0:
                for t in uB:
                    nc.vector.memset(t[:, 0:C_IN], 0.0)
            if fuse and sc0:
                for t in knB:
                    nc.vector.memset(t[:, 0:1], 0.0)
                    if fsplit:
                        nc.vector.memset(
                            t[:, C_IN + NH : 2 * C_IN + NH], 0.0
                        )

            def gx_tile(step):
                t = gxp.tile([BL, 3 * NF], FP16, tag="gx", name="gx")
                nc.sync.dma_start(
                    out=t[:],
                    in_=gx_in[:, step * 3 * NF : (step + 1) * 3 * NF],
                )
                return t

            gx_cur = gx_tile(0) if gexp else None
            gx_next = None

            nc.sync.dma_start(out=gS[:], in_=g_in[:])
            nc.sync.dma_start(out=b1S[:], in_=b1_in[:])
            nc.sync.dma_start(out=w1zS[:], in_=w1z_in[:])
            nc.sync.dma_start(out=w1zH[:], in_=w1zh_in[:])
            nc.sync.dma_start(out=w2S[:], in_=w2_in[:])
            nc.sync.dma_start(out=b2S[:], in_=b2_in[:])
            nc.sync.dma_start(out=onesS[:], in_=ones_in[:])
            nc.sync.dma_start(out=idS[:], in_=id_in[:])
            nc.sync.dma_start(out=ones32S[:], in_=ones32_in[:])
            nc.sync.dma_start(out=id6S[:], in_=id6_in[:])
            nc.sync.dma_start(out=zall[:, 0:BL], in_=z0t_in[:])
            nc.sync.dma_start(out=zs_out[:, 0:BL], in_=z0t_in[:].bitcast(F32))

            # h PSUM: one bank, 4 rotating [128,128] stage slots. Emission
            # order guarantees at most one open accumulation group at a time.
            hP = ph.tile([MLP_H, 4 * BL], F32, name="hP")

            def h_slot(step, s):
                i = (4 * step + s) % 4
                return hP[:, i * BL : (i + 1) * BL]

            CLS = (0, 1, 1, 2)
            KH_A = (1.0 / 6.0, 0.5, 0.25, 0.5)

            def zT_sl(step):
                return zall[:, step * BL : (step + 1) * BL]

            def bt_tile(step):
                t = btp.tile([1, 3 * MLP_H], F32, tag="bt", name="bt")
                nc.sync.dma_start(
                    out=t[:],
                    in_=b1t_in[:, step * 3 * MLP_H : (step + 1) * 3 * MLP_H],
                )
                return t

            bt_cur = bt_tile(0) if bias_mm else None
            bt_next = None

            # step 0 slice-0 big (no k correction at the very first stage)
            nc.tensor.matmul(
                h_slot(0, 0), lhsT=w1zS[:], rhs=zT_sl(0), start=True,
                stop=not bias_mm,
            )
            if bias_mm:
                nc.tensor.matmul(
                    h_slot(0, 0), lhsT=bt_cur[:, 0:MLP_H], rhs=ones32S[:],
                    start=False, stop=True, skip_group_check=True,
                )

            acc_nat = None   # kn1+kn2 (+kn3) natural-layout partial RK4 sum
            acc2T6 = None    # (acc_nat at s=2).T / 6 in PSUM
            zsum = None      # zT + acc2T6, f32r (state update staging)
            kt4P = None      # k4~.T PSUM
            ksP = None       # k~_s.T PSUM for next stage's kh
            kh0 = None       # delta-z^T fp16 (next step's h correction)

            for step in range(nstep):
                zT = zT_sl(step)
                if bias_mm and step + 1 < nstep:
                    bt_next = bt_tile(step + 1)
                if gexp and step + 1 < nstep:
                    gx_next = gx_tile(step + 1)
                for s in range(4):
                    col = step * 3 + CLS[s]
                    has_b = not (step == 0 and s == 0)
                    # ---- kh for this stage ----
                    if has_b:
                        kh = khp.tile([HID, BL], FP16, tag="kh", name="kh")
                        if s == 0:
                            # kh0 = (k4~.T)/6 + acc2T6 = delta-z^T
                            nc.vector.scalar_tensor_tensor(
                                out=kh[:],
                                in0=kt4P[:],
                                scalar=1.0 / 6.0,
                                in1=acc2T6[:],
                                op0=mybir.AluOpType.mult,
                                op1=mybir.AluOpType.add,
                            )
                            # state update z_step = zsum + (k4~.T)/6, f32r
                            nc.vector.scalar_tensor_tensor(
                                out=zT,
                                in0=kt4P[:],
                                scalar=1.0 / 6.0,
                                in1=zsum[:],
                                op0=mybir.AluOpType.mult,
                                op1=mybir.AluOpType.add,
                            )
                            nc.sync.dma_start(
                                out=zs_out[:, step * BL : (step + 1) * BL],
                                in_=zT.bitcast(F32),
                            )
                        else:
                            nc.vector.tensor_scalar_mul(kh[:], ksP[:], KH_A[s])
                        nc.tensor.matmul(
                            h_slot(step, s), lhsT=w1zH[:], rhs=kh[:],
                            start=False, stop=True,
                        )
                    # ---- relu (bias already in PSUM when bias_mm) ----
                    hS = hp.tile([MLP_H, BL], FP16, tag="hs")
                    if bias_mm:
                        nc.vector.tensor_scalar_max(hS[:], h_slot(step, s), 0.0)
                    else:
                        nc.vector.tensor_scalar(
                            hS[:], h_slot(step, s), b1S[:, col : col + 1], 0.0,
                            op0=mybir.AluOpType.add, op1=mybir.AluOpType.max,
                        )
                    # ---- mm2 (+ optional bias2), h-split halves ----
                    f_ps = pf.tile([BL, NF], F32, tag="fps")
                    if with_b2:
                        nc.tensor.matmul(
                            f_ps[:], lhsT=onesS[:], rhs=b2S[:],
                            start=True, stop=False,
                        )
                    if fuse and fsplit:
                        halves = ((0, NH), (NH, NF))
                    elif split == 2:
                        halves = ((0, 128), (128, NF))
                    elif split == 1:
                        halves = ((0, NH), (NH, NF))
                    else:
                        halves = ((0, NF),)
                    for (lo, hi) in halves:
                        nc.tensor.matmul(
                            f_ps[:, lo:hi], lhsT=hS[:], rhs=w2S[:, lo:hi],
                            start=not with_b2, stop=True,
                        )
                    # ---- big mm1 for the next stage slot (off chain) ----
                    if s < 3:
                        nc.tensor.matmul(
                            h_slot(step, s + 1), lhsT=w1zS[:], rhs=zT,
                            start=True, stop=False,
                        )
                        if bias_mm:
                            ncol = CLS[s + 1]
                            nc.tensor.matmul(
                                h_slot(step, s + 1),
                                lhsT=bt_cur[:, ncol * MLP_H : (ncol + 1) * MLP_H],
                                rhs=ones32S[:],
                                start=False, stop=False, skip_group_check=True,
                            )
                    elif step + 1 < nstep:
                        nc.tensor.matmul(
                            h_slot(step + 1, 0), lhsT=w1zS[:], rhs=zT,
                            start=True, stop=False,
                        )
                        if bias_mm:
                            nc.tensor.matmul(
                                h_slot(step + 1, 0),
                                lhsT=bt_next[:, 0:MLP_H],
                                rhs=ones32S[:],
                                start=False, stop=False, skip_group_check=True,
                            )
                    # ---- tanh / mul / reduce, pipelined across halves ----
                    fS = fp.tile([BL, NF], FP16, tag="fs")
                    u = None if fuse else up.tile([BL, NF], FP16, tag="u")
                    kn = kp.tile([BL, HID], FP16, tag="kn")
                    for (lo, hi) in halves:
                        nc.scalar.activation(
                            fS[:, lo:hi], f_ps[:, lo:hi],
                            mybir.ActivationFunctionType.Tanh,
                        )
                    if fuse and seg:
                        uP = uB[(4 * step + s) % 3]
                        f3 = fS[:].rearrange("p (h c) -> p h c", c=C_IN)
                        gv = (
                            gS[:, col * C_IN : (col + 1) * C_IN]
                            .unsqueeze(1)
                            .broadcast_to((BL, HID, C_IN))
                        )
                        scan_out = uP[:, 0:NF].rearrange(
                            "p (h c) -> p h c", c=C_IN
                        )
                        nc.vector._custom_dve(
                            mulscan, out=scan_out, in0=f3, in1=gv
                        )
                        # group-end prefix IS kn (accumulator reseeds per
                        # 8-elem page); strided fp32 view, no diff op
                        kn_ap = (
                            uP[:, 0:NF]
                            .rearrange("p (h c) -> p h c", c=C_IN)
                            [:, :, C_IN - 1 : C_IN]
                        )
                        kn2d = kn_ap.rearrange("p h one -> p (h one)")
                        kn = None
                    elif fuse and sc0:
                        kP = knB[(4 * step + s) % 3]
                        f3 = fS[:].rearrange("p (h c) -> p h c", c=C_IN)
                        gv = (
                            gS[:, col * C_IN : (col + 1) * C_IN]
                            .unsqueeze(1)
                            .broadcast_to((BL, HID, C_IN))
                        )
                        # stride-0 inner out: the 8 writes of each group hit
                        # one address; the last (group-end prefix) survives
                        scan_out = (
                            kP[:, 1 : 1 + HID]
                            .unsqueeze(2)
                            .broadcast_to((BL, HID, C_IN))
                        )
                        nc.vector._custom_dve(
                            mulscan, out=scan_out, in0=f3, in1=gv
                        )
                        with nc.allow_low_precision("k diff"):
                            nc.vector.tensor_tensor(
                                out=kn[:],
                                in0=kP[:, 1 : 1 + HID],
                                in1=kP[:, 0:HID],
                                op=mybir.AluOpType.subtract,
                            )
                    elif fuse:
                        uP = uB[(4 * step + s) % 3]
                        v = uP[:].rearrange("p (g c) -> p g c", c=C_IN)
                        cg = CLS[s]
                        for hi_, (lo, hi) in enumerate(halves):
                            nh = hi - lo
                            uoff = C_IN + lo + hi_ * (C_IN if fsplit else 0)
                            scan_out = uP[:, uoff : uoff + nh].rearrange(
                                "p (h c) -> p h c", c=C_IN
                            )
                            f3 = fS[:, lo:hi].rearrange(
                                "p (h c) -> p h c", c=C_IN
                            )
                            if gexp:
                                gv = gx_cur[
                                    :, cg * NF + lo : cg * NF + hi
                                ].rearrange("p (h c) -> p h c", c=C_IN)
                            else:
                                gv = (
                                    gS[:, col * C_IN : (col + 1) * C_IN]
                                    .unsqueeze(1)
                                    .broadcast_to((BL, nh // C_IN, C_IN))
                                )
                            nc.vector._custom_dve(
                                mulscan, out=scan_out, in0=f3, in1=gv
                            )
                            # kn[h] = prefix[8h+7]-prefix[8(h-1)+7] via the
                            # zero-padded leading group of each half
                            glo = uoff // C_IN - 1
                            hlo, hn = lo // C_IN, nh // C_IN
                            with nc.allow_low_precision("k diff"):
                                nc.vector.tensor_tensor(
                                    out=kn[:, hlo : hlo + hn].unsqueeze(2),
                                    in0=v[:, glo + 1 : glo + 1 + hn,
                                          C_IN - 1 : C_IN],
                                    in1=v[:, glo : glo + hn,
                                          C_IN - 1 : C_IN],
                                    op=mybir.AluOpType.subtract,
                                )
                    else:
                        for (lo, hi) in halves:
                            hlo, hhi = lo // C_IN, hi // C_IN
                            f3 = fS[:, lo:hi].rearrange("p (h c) -> p h c", c=C_IN)
                            u3 = u[:, lo:hi].rearrange("p (h c) -> p h c", c=C_IN)
                            gv = (
                                gS[:, col * C_IN : (col + 1) * C_IN]
                                .unsqueeze(1)
                                .broadcast_to((BL, hhi - hlo, C_IN))
                            )
                            nc.vector.tensor_tensor(
                                out=u3, in0=f3, in1=gv, op=mybir.AluOpType.mult
                            )
                            with nc.allow_low_precision("k reduce"):
                                nc.vector.tensor_reduce(
                                    kn[:, hlo:hhi], u3, axis=mybir.AxisListType.X,
                                    op=mybir.AluOpType.add,
                                )
                    # ---- transpose + RK4 bookkeeping ----
                    tr_lhs = kn2d if (fuse and seg) else kn[:]
                    if s < 3:
                        ksP = pkt.tile([HID, BL], KTD, tag="kt", name="kt")
                        nc.tensor.matmul(
                            ksP[:], lhsT=tr_lhs, rhs=idS[:], is_transpose=True,
                            start=True, stop=True,
                        )
                        if s == 0:
                            # acc3 holds the rank-3 AP form for seg mode
                            acc3 = kn_ap if (fuse and seg) else None
                            acc_nat = None if (fuse and seg) else kn
                        else:
                            acc_new = ap.tile([BL, HID], KTD, tag="an",
                                              name="an")
                            if fuse and seg:
                                nc.vector.tensor_tensor(
                                    out=acc_new[:].unsqueeze(2),
                                    in0=acc3,
                                    in1=kn_ap,
                                    op=mybir.AluOpType.add,
                                )
                                acc3 = acc_new[:].unsqueeze(2)
                            else:
                                nc.vector.tensor_tensor(
                                    out=acc_new[:], in0=acc_nat[:], in1=kn[:],
                                    op=mybir.AluOpType.add,
                                )
                            acc_nat = acc_new
                        if s == 2:
                            # off-chain: acc2T6 = acc_nat.T / 6 (via scaled
                            # identity), staged to SBUF so the s=0 STT has a
                            # single PSUM operand; zsum = zT + acc2T6
                            a26P = pa26.tile([HID, BL], F32, tag="a26",
                                             name="a26")
                            a26_lhs = acc_nat[:]
                            # regular matmul: transpose datapath would ignore
                            # the scaled identity's values
                            nc.tensor.matmul(
                                a26P[:], lhsT=a26_lhs, rhs=id6S[:],
                                start=True, stop=True,
                            )
                            acc2T6 = zsp.tile([HID, BL], FP16, tag="a26s",
                                              name="a26s")
                            nc.vector.tensor_copy(acc2T6[:], a26P[:])
                            zsum = zsp.tile([HID, BL], F32R, tag="zsum",
                                            name="zsum")
                            nc.vector.tensor_tensor(
                                out=zsum[:], in0=zT, in1=acc2T6[:],
                                op=mybir.AluOpType.add,
                            )
                    else:
                        kt4P = pkt.tile([HID, BL], KTD, tag="kt", name="kt4")
                        nc.tensor.matmul(
                            kt4P[:], lhsT=tr_lhs, rhs=idS[:],
                            is_transpose=True, start=True, stop=True,
                        )
                        bt_cur = bt_next
                        gx_cur = gx_next

            # final z update (last grid point)
            nc.vector.scalar_tensor_tensor(
                out=zT_sl(nstep),
                in0=kt4P[:],
                scalar=1.0 / 6.0,
                in1=zsum[:],
                op0=mybir.AluOpType.mult,
                op1=mybir.AluOpType.add,
            )
            nc.sync.dma_start(
                out=zs_out[:, nstep * BL : (nstep + 1) * BL],
                in_=zT_sl(nstep).bitcast(F32),
            )
    import sys

    print(f"[kernel] tile trace+schedule: {_time.time()-t0:.1f}s", file=sys.stderr)
    t1 = _time.time()
    nc.finalize()
    print(f"[kernel] finalize: {_time.time()-t1:.1f}s", file=sys.stderr)
    return nc


def _get_nc(nstep: int, with_b2: bool):
    key = (nstep, with_b2) + _flags()
    if key not in _CACHE:
        _CACHE[key] = _build(nstep, with_b2)
    return _CACHE[key]


def _host_prep(coeffs, Wi1, bi1, Wi2, bi2, W1, b1, W2, b2, nstep: int):
    coeffs = np.asarray(coeffs, dtype=np.float32)
    a = coeffs[:, :, 0:8]
    b = coeffs[:, :, 8:16]
    c = coeffs[:, :, 16:24]
    d = coeffs[:, :, 24:32]

    X0 = a[:, 0]
    z0 = np.tanh(
        np.maximum(X0 @ Wi1 + bi1, 0.0).astype(np.float32) @ Wi2 + bi2
    ).astype(np.float32)

    g = np.empty((B, nstep, 3, C_IN), dtype=np.float32)
    g[:, :, 0] = b[:, :nstep]
    g[:, :, 1] = 2.0 * b[:, :nstep] + 2.0 * c[:, :nstep] + 1.5 * d[:, :nstep]
    last = NSTEP - 1  # 126 in full problem
    for i in range(nstep):
        if i < last:
            g[:, i, 2] = b[:, i + 1]
        else:
            g[:, i, 2] = b[:, i] + 2.0 * c[:, i] + 3.0 * d[:, i]
    g16 = g.reshape(B, nstep * 3 * C_IN).astype(np.float16)
    gexp_on = _flags()[5]
    gx16 = (np.broadcast_to(
        g.astype(np.float16)[:, :, :, None, :], (B, nstep, 3, HID, C_IN)
    ).reshape(B, nstep * 3 * NF) if gexp_on else None)

    tcols = np.empty((nstep, 3), dtype=np.float32)
    tcols[:, 0] = np.arange(nstep, dtype=np.float32)
    tcols[:, 1] = tcols[:, 0] + 0.5
    tcols[:, 2] = tcols[:, 0] + 1.0
    bias1 = (
        b1[None, None, :] + tcols[:, :, None] * W1[0][None, None, :]
    ).astype(np.float32)
    bias1 = bias1.reshape(nstep * 3, MLP_H).T.copy()  # [128, nstep*3]

    f16_big = _flags()[0]
    bias1t = np.ascontiguousarray(bias1.T.reshape(1, -1))
    shared = {
        "bias1": bias1,
        "bias1t": bias1t,
        "ones32": np.ones((1, BL), dtype=np.float32),
        "w1z": np.ascontiguousarray(
            W1[1:], dtype=(np.float16 if f16_big else np.float32)
        ),
        "w1zh": np.ascontiguousarray(W1[1:], dtype=np.float16),
        "w2": np.ascontiguousarray(W2, dtype=np.float16),
        "b2r": np.ascontiguousarray(b2[None, :], dtype=np.float16),
        "onesr": np.ones((1, BL), dtype=np.float16),
        "ident": np.eye(
            BL, dtype=(np.float32 if _flags()[6] else np.float16)
        ),
        "ident6": (np.eye(BL) / 6.0).astype(
            np.float32 if _flags()[6] else np.float16
        ),
    }
    in_maps = []
    for core in range(NCORES):
        sl = slice(core * BL, (core + 1) * BL)
        m = dict(shared)
        m["g"] = np.ascontiguousarray(g16[sl])
        m["gexp"] = (np.ascontiguousarray(gx16[sl]) if gexp_on
                     else np.zeros((BL, 1), dtype=np.float16))
        m["z0t"] = np.ascontiguousarray(z0[sl].T)
        in_maps.append(m)
    return in_maps, z0


def kernel(coeffs, Wi1, bi1, Wi2, bi2, W1, b1, W2, b2, _nstep: int = NSTEP,
           _trace: bool = False):
    import time as _time
    import sys

    nstep = _nstep
    with_b2 = bool(np.any(np.asarray(b2)))
    nc = _get_nc(nstep, with_b2)
    in_maps, _ = _host_prep(
        coeffs, Wi1, bi1, Wi2, bi2, W1, b1, W2, b2, nstep
    )
    t0 = _time.time()
    res = run_bass_kernel_spmd(nc, in_maps, list(range(NCORES)), trace=_trace)
    print(f"[kernel] spmd run (compile+exec): {_time.time()-t0:.1f}s", file=sys.stderr)
    out = np.empty((B, nstep + 1, HID), dtype=np.float32)
    for core in range(NCORES):
        zs = res.results[core]["zs"].reshape(HID, nstep + 1, BL)
        out[core * BL : (core + 1) * BL] = zs.transpose(2, 1, 0)
    if _trace:
        kernel.last_results = res
    return out
